# revision 17
# baseline (speedup 1.0000x reference)
"""Trainium2 Bass kernel: multi-head attention with RoPE + gated prompt
injection (nn_Attention_28080496181816), sharded over 8 NeuronCores.

Sharding: tensor-parallel over heads. Core c owns heads [4c, 4c+4):
  - wq/wk/wv column-sharded (per-head), o-proj via AllGather of the
    per-core attention outputs + column-sharded wo matmul.
  - Host-side unshard is a pure concatenation of output column slices.

Layout: "T-major" — activations live as [feature, token] on device so
every matmul contraction lands on the partition axis with no on-device
transposes. RoPE pairs are made contiguous by permuting wq/wk rows
(per head: even hd dims then odd hd dims) on the host.

Scheduling: the PE clock ramps with continuous execution (full speed
only after 3us without a stall), so emission is organized as a dense
stream of 8-matmul projection "quanta" with the ACT-bound attention
units threaded between them.  Attention for a (batch, query-group)
starts as soon as that chunk's Q/K are projected+RoPE'd; per-(head,qg)
1MB AllGathers fire immediately after each unit; o-proj accumulation
is gated per-head so the tail never waits on a full-batch collective.
"""

import math
import os
import sys
import types
from collections import deque

import numpy as np
import ml_dtypes

# --- optional NTFF profile hook shim (only needed if BASS_TRACE is set;
# the stock image lacks antenv.axon_hooks) ---
try:
    import antenv.axon_hooks  # noqa: F401
except Exception:
    try:
        import antenv
        _m = types.ModuleType("antenv.axon_hooks")
        _hook = [None]
        _m.set_axon_ntff_profile_hook = lambda h: _hook.__setitem__(0, h)
        _m.get_axon_ntff_profile_hook = lambda: _hook[0]
        sys.modules["antenv.axon_hooks"] = _m
        antenv.axon_hooks = _m
        from trn_agent_boot.trn_boot import _ntff_profile_via_ctypes
        _p = _ntff_profile_via_ctypes("/opt/axon/libaxon_pjrt.so")
        if _p is not None:
            _m.set_axon_ntff_profile_hook(_p)
    except Exception:
        pass

import concourse.bacc as bacc
import concourse.mybir as mybir
import concourse.tile as tile
from concourse import bass_utils

_orig_upload = bass_utils.upload_artifacts


def _safe_upload(tmpdir):
    try:
        return _orig_upload(tmpdir)
    except Exception:
        return tmpdir


bass_utils.upload_artifacts = _safe_upload

BF16 = mybir.dt.bfloat16
F32 = mybir.dt.float32
NPBF16 = ml_dtypes.bfloat16

B, S, D, H, HD, PL = 2, 1024, 4096, 32, 128, 10
NC = 8              # cores
HLOC = H // NC      # 4 heads per core
DLOC = HLOC * HD    # 512
T = B * S           # 2048
NDX = D // 128      # 32 contraction blocks
NQT = S // 128      # 8 query tiles per batch
NQG = NQT // 4      # 2 query groups of 512
SCALE = 1.0 / math.sqrt(HD)

_PROG_CACHE = {}


def _analyze_mask(mask):
    """Classify each 128x128 tile of the additive mask: skip (fully
    masked), clear (all zero) or mixed (ship the transposed, pre-scaled
    tile). Deduplicates mixed tiles."""
    mq = np.asarray(mask).reshape(S, S)
    plan = []
    uniq = {}
    mlist = []
    for qi in range(NQT):
        row = []
        for kb in range(NQT):
            sub = mq[qi * 128:(qi + 1) * 128, kb * 128:(kb + 1) * 128]
            if np.all(sub <= -1e8):
                continue
            if np.all(sub == 0):
                row.append((kb, None))
                continue
            tt = np.ascontiguousarray(sub.T.astype(np.float32) / SCALE)
            key = tt.tobytes()
            if key not in uniq:
                uniq[key] = len(mlist)
                mlist.append(tt)
            row.append((kb, uniq[key]))
        plan.append(row)
    return plan, mlist


def _group_plan(plan, n_mtiles):
    """512-wide query groups. Per group: list of (kb, q0, q1, adds) with
    q0..q1 the covered query quarters and adds = [(quarter, mtile_idx)];
    mtile_idx == n_mtiles selects the -inf tile. The first kb of each
    group always spans the full group so PSUM has_written is set."""
    NEG = n_mtiles
    plan2 = []
    for qg in range(NQG):
        qmode = []
        for q in range(4):
            qmode.append(dict(plan[qg * 4 + q]))
        live = sorted(set().union(*[set(d.keys()) for d in qmode]))
        entries = []
        for j, kb in enumerate(live):
            pres = [kb in qmode[q] for q in range(4)]
            if j == 0:
                q0, q1 = 0, 3
            else:
                q0 = min(q for q in range(4) if pres[q])
                q1 = max(q for q in range(4) if pres[q])
            adds = []
            for q in range(q0, q1 + 1):
                if not pres[q]:
                    adds.append((q, NEG))
                elif qmode[q][kb] is not None:
                    adds.append((q, qmode[q][kb]))
            entries.append((kb, q0, q1, tuple(adds)))
        plan2.append(tuple(entries))
    return plan2


def _build_program(plan2, n_mt):
    """Build + compile the SPMD program (identical on all 8 cores).
    n_mt counts mask tiles INCLUDING the trailing -inf tile."""
    nc = bacc.Bacc("TRN2", target_bir_lowering=False, debug=False, num_devices=NC)

    # p-major host layouts so each logical group is ONE big DMA
    xt = nc.dram_tensor("xt", [4, 128, NDX, 512], BF16, kind="ExternalInput")
    wqt = nc.dram_tensor("wqt", [HLOC, 128, NDX, 128], BF16, kind="ExternalInput")
    wkt = nc.dram_tensor("wkt", [HLOC, 128, NDX, 128], BF16, kind="ExternalInput")
    wvt = nc.dram_tensor("wvt", [128, NDX, DLOC], BF16, kind="ExternalInput")
    wot = nc.dram_tensor("wot", [128, NDX, DLOC], BF16, kind="ExternalInput")
    pt = nc.dram_tensor("pt", [128, NDX, PL], BF16, kind="ExternalInput")
    # [128, S] duplicated-half rope tables: cs2 = [cos; cos], sn2 = [-sin; sin]
    cs2d = nc.dram_tensor("cs2", [128, S], F32, kind="ExternalInput")
    sn2d = nc.dram_tensor("sn2", [128, S], F32, kind="ExternalInput")
    gates = nc.dram_tensor("gates", [PL, HLOC], F32, kind="ExternalInput")
    mtiles = nc.dram_tensor("mtiles", [n_mt, 128, 128], BF16, kind="ExternalInput")
    ident = nc.dram_tensor("ident", [128, 128], BF16, kind="ExternalInput")
    out_d = nc.dram_tensor("out", [T, DLOC], F32, kind="ExternalOutput")

    AF = mybir.ActivationFunctionType
    OP = mybir.AluOpType
    labels = {}
    nc._unit_labels = labels

    def _lb(inst, tag):
        labels[inst.ins.name] = tag
        return inst

    with tile.TileContext(nc) as tc:
        with (
            tc.tile_pool(name="const", bufs=1) as cpool,
            tc.tile_pool(name="wres", bufs=1) as wres,
            tc.tile_pool(name="stream", bufs=1) as sp,
            tc.tile_pool(name="act", bufs=1) as ap,
            tc.tile_pool(name="psum", bufs=1, space="PSUM") as pp,
            tc.tile_pool(name="dram", bufs=1, space="DRAM") as dp,
        ):
            # ---- persistent constants / weights ----
            # small tables go on the (otherwise idle at start) gpsimd queue
            gates_sb = cpool.tile([PL, HLOC], F32, tag="gates")
            nc.gpsimd.dma_start(gates_sb[:], gates[:])
            mt_sb = []
            for i in range(n_mt):
                t = cpool.tile([128, 128], BF16, tag=f"mt{i}", name=f"mt{i}")
                nc.gpsimd.dma_start(t[:], mtiles[i])
                mt_sb.append(t)
            id_sb = cpool.tile([128, 128], BF16, tag="ident")
            nc.gpsimd.dma_start(id_sb[:], ident[:])
            ones_col = cpool.tile([128, 1], BF16, tag="ones_col")
            nc.vector.memset(ones_col[:], 1.0)
            cs2 = cpool.tile([128, S], F32, tag="cs2")
            sn2 = cpool.tile([128, S], F32, tag="sn2")

            def emit_big_consts():
                # big rope tables + prompt, after the startup-critical DMAs
                nc.sync.dma_start(pt_sb[:], pt[:])
                nc.sync.dma_start(cs2[:], cs2d[:])
                nc.sync.dma_start(sn2[:], sn2d[:])

            # wv / wo resident: 4 tiles each of [128, 8*512]
            wv_sb = [wres.tile([128, 8 * DLOC], BF16, tag=f"wv{j}",
                               name=f"wv{j}") for j in range(4)]
            wo_sb = [wres.tile([128, 8 * DLOC], BF16, tag=f"wo{j}",
                               name=f"wo{j}") for j in range(4)]

            def emit_wvwo_loads():
                for j in range(4):
                    nc.sync.dma_start(wv_sb[j][:], wvt[:, 8 * j:8 * (j + 1), :])
                for j in range(4):
                    nc.gpsimd.dma_start(wo_sb[j][:], wot[:, 8 * j:8 * (j + 1), :])

            def wv_sl(i):
                return wv_sb[i // 8][:, (i % 8) * DLOC:(i % 8 + 1) * DLOC]

            def wo_sl(i):
                return wo_sb[i // 8][:, (i % 8) * DLOC:(i % 8 + 1) * DLOC]

            pt_sb = cpool.tile([128, NDX * PL], BF16, tag="pt")

            pk_sb = [ap.tile([128, PL], BF16, tag=f"pk{h}", name=f"pk{h}")
                     for h in range(HLOC)]
            pv_sb = ap.tile([PL, DLOC], BF16, tag="pv")

            # per-(batch, head, query-group) AllGather buffers (1MB each)
            agin = {}
            agout = {}
            for b in range(B):
                for h in range(HLOC):
                    for qg in range(NQG):
                        agin[b, h, qg] = dp.tile(
                            [128, 512], BF16,
                            tag=f"agin{b}_{h}_{qg}", name=f"agin{b}_{h}_{qg}")
                        agout[b, h, qg] = dp.tile(
                            [NC, 128, 512], BF16,
                            tag=f"agout{b}_{h}_{qg}", name=f"agout{b}_{h}_{qg}",
                            addr_space="Shared")

            XT_BUFS = 5     # [128, 4096] quarters (one chunk + 1 prefetch)
            WQK_BUFS = 2
            QK_BUFS = 5
            V_BUFS = 10
            AGT_BUFS = 5

            qT = {}
            kT = {}
            v_sb = {}
            xts_of = {}

            def x_quarters(tcg):
                """Allocate + DMA the 4 x-quarters of chunk tcg, spread
                over 4 queues."""
                xts = [sp.tile([128, 8 * 512], BF16, tag="xt",
                               bufs=XT_BUFS, name=f"xt{tcg}_{q}")
                       for q in range(4)]
                engs = [nc.sync, nc.gpsimd, nc.sync, nc.gpsimd]
                for q in range(4):
                    engs[q].dma_start(xts[q][:],
                                      xt[tcg, :, 8 * q:8 * (q + 1), :])
                xts_of[tcg] = xts

            def qkv_chunk_items(b, tc2):
                """List of ('Q'|'X', closure) items for one 512-token
                chunk: k-proj (h0..h3), q-proj (h0..h3), v-proj (t0..t3).
                'Q' items emit 8 matmuls; 'X' items are free (DMA issue /
                allocs). Everything is deferred — emission happens only
                when the closure runs, in list order."""
                tcg = b * 2 + tc2
                cols = slice(tc2 * 512, (tc2 + 1) * 512)

                def x_sl(i):
                    xts = xts_of[tcg]
                    return xts[i // 8][:, (i % 8) * 512:(i % 8 + 1) * 512]

                def prologue():
                    if tc2 == 0:
                        qT[b] = [sp.tile([128, S], BF16, tag="qT",
                                         bufs=QK_BUFS, name=f"qT{b}_{j}")
                                 for j in range(HLOC)]
                        kT[b] = [sp.tile([128, S], BF16, tag="kT",
                                         bufs=QK_BUFS, name=f"kT{b}_{j}")
                                 for j in range(HLOC)]
                        v_sb[b] = [sp.tile([128, DLOC], BF16, tag="v",
                                           bufs=V_BUFS, name=f"v{b}_{j}")
                                   for j in range(NQT)]

                def prefetch_next():
                    if tcg + 1 < 4:
                        x_quarters(tcg + 1)

                st = {}  # chain-id -> (wt, ps) filled at closure run time
                chains = []  # (cid, fetch_or_None, [quanta], tail_or_None)
                for proj, wdram, dstT in ((1, wkt, kT), (0, wqt, qT)):
                    for dqb in range(HLOC):
                        cid = (proj, dqb)

                        def fetch(cid=cid, wdram=wdram):
                            wt = sp.tile([128, NDX * 128], BF16, tag="wqk",
                                         bufs=WQK_BUFS)
                            dqb = cid[1]
                            nc.scalar.dma_start(wt[:, 0:2048],
                                                wdram[dqb, :, 0:16, :])
                            nc.scalar.dma_start(wt[:, 2048:4096],
                                                wdram[dqb, :, 16:32, :])
                            st[cid] = wt

                        quanta = []
                        for i0 in range(0, NDX, 8):
                            def quantum(i0=i0, cid=cid, proj=proj, dqb=dqb):
                                if i0 == 0:
                                    st[cid, "ps"] = pp.tile(
                                        [128, 512], F32, tag="mm512", bufs=2,
                                        name=f"ps{b}_{tc2}_{proj}_{dqb}")
                                wt = st[cid]
                                ps = st[cid, "ps"]
                                for i in range(i0, i0 + 8):
                                    _lb(nc.tensor.matmul(
                                        ps[:], wt[:, i * 128:(i + 1) * 128],
                                        x_sl(i), start=(i == 0),
                                        stop=(i == NDX - 1)),
                                        f"qkv{b}.{tc2}.p{proj}.d{dqb}.{i}")
                            quanta.append(quantum)

                        def tail(cid=cid, proj=proj, dqb=dqb, dstT=dstT):
                            if proj == 1 and b == 0 and tc2 == 0:
                                # prompt keys, reusing this head's wk tiles
                                wt = st[cid]
                                psk = pp.tile([128, 512], F32, tag="sc", bufs=2)
                                for i in range(NDX):
                                    nc.tensor.matmul(
                                        psk[:, 0:PL],
                                        wt[:, i * 128:(i + 1) * 128],
                                        pt_sb[:, i * PL:(i + 1) * PL],
                                        start=(i == 0), stop=(i == NDX - 1))
                                nc.vector.tensor_copy(pk_sb[dqb][:],
                                                      psk[:, 0:PL])
                            # fused RoPE: ps rows 0:64 = even half (real),
                            # rows 64:128 = odd half (imag)
                            ps = st[cid, "ps"]
                            c_sl = cs2[:, cols]
                            s_sl = sn2[:, cols]
                            t1 = sp.tile([128, 512], BF16, tag="rt", bufs=2)
                            t2 = sp.tile([128, 512], BF16, tag="rt", bufs=2)
                            nc.vector.tensor_tensor(t1[:], ps[:], c_sl,
                                                    op=OP.mult)
                            # t2[0:64] = -sin*imag ; t2[64:128] = sin*real
                            nc.vector.tensor_tensor(t2[0:64, :], ps[64:128, :],
                                                    s_sl[0:64, :], op=OP.mult)
                            nc.vector.tensor_tensor(t2[64:128, :], ps[0:64, :],
                                                    s_sl[64:128, :], op=OP.mult)
                            nc.gpsimd.tensor_tensor(dstT[b][dqb][:, cols],
                                                    t1[:], t2[:], op=OP.add)
                        chains.append((fetch, quanta, tail))

                for tblk in range(4):
                    cid = ("v", tblk)
                    quanta = []
                    for i0 in range(0, NDX, 8):
                        def quantum(i0=i0, cid=cid, tblk=tblk):
                            if i0 == 0:
                                st[cid, "ps"] = pp.tile(
                                    [128, 512], F32, tag="mm512", bufs=2,
                                    name=f"psv{b}_{tc2}_{tblk}")
                            ps = st[cid, "ps"]
                            for i in range(i0, i0 + 8):
                                _lb(nc.tensor.matmul(
                                    ps[:],
                                    x_sl(i)[:, tblk * 128:(tblk + 1) * 128],
                                    wv_sl(i), start=(i == 0),
                                    stop=(i == NDX - 1)),
                                    f"v{b}.{tc2}.{tblk}.{i}")
                        quanta.append(quantum)

                    def tail(cid=cid, tblk=tblk):
                        nc.vector.tensor_copy(v_sb[b][tc2 * 4 + tblk][:],
                                              st[cid, "ps"][:])
                    chains.append((None, quanta, tail))

                # flatten with weight fetches pipelined 3 quanta ahead
                items = [("X", prologue)]
                if chains[0][0] is not None:
                    items.append(("X", chains[0][0]))
                for ci, (fetch, quanta, tail) in enumerate(chains):
                    if ci == 8:
                        # prefetch next chunk's x late (short WAR park)
                        items.append(("X", prefetch_next))
                    for j, qu in enumerate(quanta):
                        items.append(("Q", qu))
                        if j == 0 and ci + 1 < len(chains) \
                                and chains[ci + 1][0] is not None:
                            items.append(("X", chains[ci + 1][0]))
                    if tail is not None:
                        items.append(("X", tail))

                if b == 0 and tc2 == 0:
                    def pv_quantum():
                        psv = pp.tile([128, 512], F32, tag="sc", bufs=2)
                        for i in range(NDX):
                            nc.tensor.matmul(psv[0:PL, :],
                                             pt_sb[:, i * PL:(i + 1) * PL],
                                             wv_sl(i),
                                             start=(i == 0),
                                             stop=(i == NDX - 1))
                        nc.vector.tensor_copy(pv_sb[:], psv[0:PL, :])
                        for hh in range(HLOC):
                            nc.vector.tensor_scalar(
                                pv_sb[0:PL, hh * 128:(hh + 1) * 128],
                                pv_sb[0:PL, hh * 128:(hh + 1) * 128],
                                gates_sb[0:PL, hh:hh + 1], None, op0=OP.mult)
                    items.append(("Q", pv_quantum))
                return items

            # ---------------- attention unit ----------------
            unit_idx = [0]

            def att_unit(b, h, qg, fill):
                stage = sp.tile([128, 512], BF16, tag="stage", bufs=2,
                                name=f"stage{b}_{h}_{qg}")
                qbase = qg * 512
                entries = plan2[qg]
                probs = []
                for j, (kb, q0, q1, adds) in enumerate(entries):
                    coff = q0 * 128
                    ncols = (q1 - q0 + 1) * 128
                    ssc = pp.tile([128, 512], F32, tag="sc", bufs=2)
                    _lb(nc.tensor.matmul(
                        ssc[:, coff:coff + ncols],
                        kT[b][h][:, kb * 128:(kb + 1) * 128],
                        qT[b][h][:, qbase + coff:qbase + coff + ncols],
                        start=True, stop=(not adds)),
                        f"sc{b}.h{h}.g{qg}.k{kb}")
                    for ai, (q, idx) in enumerate(adds):
                        nc.tensor.matmul(
                            ssc[:, q * 128:(q + 1) * 128], id_sb[:],
                            mt_sb[idx][:], start=False,
                            stop=(ai == len(adds) - 1))
                    pr = sp.tile([128, 512], BF16, tag="probs", bufs=7)
                    nc.scalar.activation(pr[:, coff:coff + ncols],
                                         ssc[:, coff:coff + ncols],
                                         AF.Exp, scale=SCALE)
                    probs.append((kb, coff, ncols, pr))
                    if j % 2 == 1:
                        fill(1)
                # prompt scores
                psc = pp.tile([128, 512], F32, tag="sc", bufs=2)
                nc.tensor.matmul(psc[0:PL, :], pk_sb[h][:],
                                 qT[b][h][:, qbase:qbase + 512],
                                 start=True, stop=True)
                ppr = sp.tile([PL, 512], BF16, tag="pprobs", bufs=1)
                nc.scalar.activation(ppr[:], psc[0:PL, :], AF.Exp,
                                     scale=SCALE)
                fill(2)
                # PV accumulation + sums
                po = pp.tile([128, 512], F32, tag="pv", bufs=3)
                pss = pp.tile([128, 512], F32, tag="aux", bufs=1)
                n = len(probs)
                for i, (kb, coff, ncols, pr) in enumerate(probs):
                    _lb(nc.tensor.matmul(
                        po[:, coff:coff + ncols],
                        v_sb[b][kb][:, h * 128:(h + 1) * 128],
                        pr[:, coff:coff + ncols],
                        start=(i == 0), stop=(i == n - 1)),
                        f"pv{b}.h{h}.g{qg}.k{kb}")
                for i, (kb, coff, ncols, pr) in enumerate(probs):
                    nc.tensor.matmul(
                        pss[0:1, coff:coff + ncols], ones_col[:, 0:1],
                        pr[:, coff:coff + ncols],
                        start=(i == 0), stop=(i == n - 1))
                ppo = pp.tile([128, 512], F32, tag="pv", bufs=3)
                nc.tensor.matmul(ppo[:], pv_sb[0:PL, h * 128:(h + 1) * 128],
                                 ppr[:], start=True, stop=True)
                nc.tensor.matmul(pss[32:33, :], ones_col[0:PL, 0:1], ppr[:],
                                 start=True, stop=True)
                # reciprocals on the (lightly loaded) DVE; bf16 out is
                # fine — the denominators only normalize probabilities
                recs = sp.tile([1, 1024], BF16, tag="recs", bufs=1)
                with nc.allow_low_precision(reason="softmax denom in bf16"):
                    nc.vector.reciprocal(recs[0:1, 0:512], pss[0:1, :])
                    nc.vector.reciprocal(recs[0:1, 512:1024], pss[32:33, :])
                # broadcast row-vector across partitions (GpSimd)
                bcs = sp.tile([128, 1024], BF16, tag="bcs", bufs=1)
                nc.gpsimd.partition_broadcast(bcs[:], recs[0:1, :])
                fill(1)
                # normalize + combine: DVE reads PSUM directly
                t1 = sp.tile([128, 512], BF16, tag="cmb", bufs=2)
                t2 = sp.tile([128, 512], BF16, tag="cmb", bufs=2)
                nc.vector.tensor_tensor(t1[:], po[:], bcs[:, 0:512], op=OP.mult)
                nc.vector.tensor_tensor(t2[:], ppo[:], bcs[:, 512:1024], op=OP.mult)
                nc.gpsimd.tensor_tensor(stage[:], t1[:], t2[:], op=OP.add)
                unit_idx[0] += 1
                nc.gpsimd.dma_start(agin[b, h, qg][:], stage[:])
                nc.gpsimd.collective_compute(
                    "AllGather", OP.bypass,
                    replica_groups=[list(range(NC))],
                    ins=[agin[b, h, qg].opt()],
                    outs=[agout[b, h, qg].opt()])

            # ---------------- output projection ----------------
            ost_state = {}

            def oproj_items(b):
                """Items for all 8 output tiles of batch b.  agt tiles
                hold a 256-token half-group per head (contiguous 512B DMA
                runs); fetches run half a group ahead and the accumulation
                is gated per head."""
                def make_fetch(hg, hl):
                    # hg = half-group index 0..3 (qg*2 + half)
                    def fetch(hg=hg, hl=hl):
                        qg, half = hg // 2, hg % 2
                        agt = sp.tile([128, NC * 256], BF16, tag="ag",
                                      bufs=AGT_BUFS, name=f"ag{b}_{hg}_{hl}")
                        nc.sync.dma_start(
                            agt[:].rearrange("p (n c) -> p n c", n=NC),
                            agout[b, hl, qg][:, :, half * 256:(half + 1) * 256]
                            .rearrange("n p c -> p n c"))
                        ost_state[b, hg, hl] = agt
                    return fetch

                def make_quantum(tq, hl):
                    def quantum(tq=tq, hl=hl):
                        if hl == 0:
                            ost_state[b, tq, "ps"] = pp.tile(
                                [128, 512], F32, tag="mm512", bufs=2,
                                name=f"pso{b}_{tq}")
                        pso = ost_state[b, tq, "ps"]
                        agt = ost_state[b, tq // 2, hl]
                        c0 = (tq % 2) * 128
                        for core in range(NC):
                            i = core * HLOC + hl
                            _lb(nc.tensor.matmul(
                                pso[:],
                                agt[:, core * 256 + c0:core * 256 + c0 + 128],
                                wo_sl(i), start=(hl == 0 and core == 0),
                                stop=(hl == 3 and core == NC - 1)),
                                f"op{b}.t{tq}.{i}")
                        if hl == 3:
                            ost = sp.tile([128, 512], F32, tag="ost", bufs=1)
                            nc.vector.tensor_copy(ost[:], pso[:])
                            r0 = b * S + tq * 128
                            nc.sync.dma_start(out_d[r0:r0 + 128, :], ost[:])
                            del ost_state[b, tq, "ps"]
                    return quantum

                items = [("X", make_fetch(0, hl)) for hl in range(HLOC)]
                for hg in range(4):
                    for sub in range(2):
                        tq = hg * 2 + sub
                        for hl in range(HLOC):
                            items.append(("Q", make_quantum(tq, hl)))
                            # prefetch next half-group per head right after
                            # this head's last reader (safe buffer recycle)
                            if sub == 1 and hg + 1 < 4:
                                items.append(("X", make_fetch(hg + 1, hl)))
                return items

            # ---------------- emission driver ----------------
            def run_pair(units, items):
                """Interleave attention units evenly into an item stream;
                units call fill(n) internally at stall-prone points."""
                q = deque(items)

                def fill(n):
                    done = 0
                    while done < n and q:
                        kind, fn = q.popleft()
                        fn()
                        done += kind == "Q"
                    # run any free items that follow
                    while q and q[0][0] == "X":
                        q.popleft()[1]()

                def nq():
                    return sum(1 for kind, _ in q if kind == "Q")

                nu = len(units)
                for i, (b, h, qg) in enumerate(units):
                    budget = max(0, (nq() - 5 * (nu - i)) // (nu - i + 1))
                    fill(budget)
                    att_unit(b, h, qg, fill)
                fill(nq())
                while q:
                    q.popleft()[1]()

            # phase 0: b0 chunk0, dense (DMA-bound startup)
            x_quarters(0)
            for j, (kind, fn) in enumerate(qkv_chunk_items(0, 0)):
                fn()
                if j == 2:
                    emit_big_consts()
                elif j == 6:
                    emit_wvwo_loads()

            run_pair([(0, h, 0) for h in range(HLOC)],
                     qkv_chunk_items(0, 1))
            run_pair([(0, h, 1) for h in range(HLOC)],
                     qkv_chunk_items(1, 0))
            run_pair([(1, h, 0) for h in range(HLOC)],
                     qkv_chunk_items(1, 1))
            run_pair([(1, h, 1) for h in range(HLOC)], oproj_items(0))
            for kind, fn in oproj_items(1):
                fn()

    nc.compile()
    return nc


def kernel(**inputs):
    x = np.asarray(inputs["x"], np.float32)
    wq = np.asarray(inputs["wq"], np.float32)
    wk = np.asarray(inputs["wk"], np.float32)
    wv = np.asarray(inputs["wv"], np.float32)
    wo = np.asarray(inputs["wo"], np.float32)
    prompt = np.asarray(inputs["prompt"], np.float32)
    prompt_gate = np.asarray(inputs["prompt_gate"], np.float32)
    freqs_cos = np.asarray(inputs["freqs_cos"], np.float32)
    freqs_sin = np.asarray(inputs["freqs_sin"], np.float32)
    mask = np.asarray(inputs["mask"], np.float32)

    plan, mlist = _analyze_mask(mask)
    plan2 = _group_plan(plan, len(mlist))
    n_mt = len(mlist) + 1  # + trailing -inf tile
    plan_key = (tuple(plan2), n_mt)
    if plan_key not in _PROG_CACHE:
        _PROG_CACHE[plan_key] = _build_program(plan2, n_mt)
    nc = _PROG_CACHE[plan_key]

    # ---- shared host prep ----
    perm = np.concatenate([np.arange(0, HD, 2), np.arange(1, HD, 2)])
    xT = np.ascontiguousarray(x.reshape(T, D).T.astype(NPBF16))
    # [4, 128, NDX, 512]: [tcg, dx_in_block, dx_block, t_in_chunk]
    xt_tiles = np.ascontiguousarray(
        xT.reshape(NDX, 128, 4, 512).transpose(2, 1, 0, 3))
    ptT = np.ascontiguousarray(prompt.T.astype(NPBF16))       # [D, PL]
    pt_tiles = np.ascontiguousarray(
        ptT.reshape(NDX, 128, PL).transpose(1, 0, 2))
    cosT = np.ascontiguousarray(freqs_cos.T.astype(np.float32))  # [64, S]
    sinT = np.ascontiguousarray(freqs_sin.T.astype(np.float32))
    cs2 = np.concatenate([cosT, cosT], axis=0)                 # [128, S]
    sn2 = np.concatenate([-sinT, sinT], axis=0)                # [128, S]
    neg = np.full((1, 128, 128), -1e30, np.float32)
    if mlist:
        mtiles = np.concatenate([np.stack(mlist), neg]).astype(NPBF16)
    else:
        mtiles = neg.astype(NPBF16)

    def shard_qk(w, c):
        rows = np.concatenate(
            [c * DLOC + j * HD + perm for j in range(HLOC)])
        wT = w[rows, :].T.astype(NPBF16)                      # [D, DLOC]
        return np.ascontiguousarray(
            wT.reshape(NDX, 128, HLOC, 128).transpose(2, 1, 0, 3))

    def shard_rhs(w, c):
        # rows c*DLOC..+DLOC of w, transposed -> [D, DLOC] -> [128,NDX,DLOC]
        wT = w[c * DLOC:(c + 1) * DLOC, :].T.astype(NPBF16)
        return np.ascontiguousarray(wT.reshape(NDX, 128, DLOC).transpose(1, 0, 2))

    in_maps = []
    for c in range(NC):
        in_maps.append(dict(
            xt=xt_tiles,
            wqt=shard_qk(wq, c),
            wkt=shard_qk(wk, c),
            wvt=shard_rhs(wv, c),
            wot=shard_rhs(wo, c),
            pt=pt_tiles,
            cs2=cs2,
            sn2=sn2,
            gates=np.ascontiguousarray(np.repeat(
                prompt_gate.reshape(H)[c * HLOC:(c + 1) * HLOC][None, :],
                PL, axis=0)).astype(np.float32),
            mtiles=mtiles,
            ident=np.eye(128, dtype=NPBF16),
        ))

    res = bass_utils.run_bass_kernel_spmd(
        nc, in_maps, core_ids=list(range(NC)),
        trace=bool(os.environ.get("BASS_TRACE")))
    kernel.last_result = res

    full = np.empty((T, D), np.float32)
    for c in range(NC):
        full[:, c * DLOC:(c + 1) * DLOC] = res.results[c]["out"]
    return full.reshape(B, S, D)
